# revision 54
# baseline (speedup 1.0000x reference)
"""Minibatch discrimination (Salimans et al. 2016) on 8 Trainium2 cores.

Reference computation:
    m = (x @ W).reshape(B, K, D)                      # [1024, 32, 8]
    L1[b1, k, b2] = sum_d |m[b1,k,d] - m[b2,k,d]|
    mb[b1, k]     = sum_b2 exp(-L1[b1, k, b2])
    out           = concat([x, mb], axis=-1)          # [1024, 2080]

Sharding: data-parallel over batch rows (128 query rows per core) for the
pairwise stage. Every core computes the FULL m^T = (x @ W)^T locally from
a host-staged x^T (bf16) and W, so no collective (and none of the ~50us
CC-stream setup + sync) is needed anywhere: the kernel is embarrassingly
parallel per core. Host-side staging only transposes/casts/replicates
inputs; all FLOPs run on device.

Per-core microkernel layout: kd=K*D=256 on partitions (2 chunks of 128),
b2=1024 on the free dim. Per query row i:
  - DVE dual-op tensor_scalar: |M_T[kd, b2] - m_T_local[kd, i]| terms in
    ONE bf16 pass per chunk (chunk0: -2*min(a,s); chunk1: relu(a-s), with
    the ACT engine taking a subset of chunk-1 passes to balance load).
  - PE matmul vs a block-diagonal ones selector sums over d (partition
    reduction kd -> k) into PSUM, 4 query rows packed per [128, 1024]
    PSUM tile; matmuls are issued j-inner so the 4 column-tiles of the
    PE array run concurrently.
  - ACT Exp(scale=-1) with fused accum_out produces sum_b2 exp(-L1) in
    one pass per 4 rows.
"""

import sys

sys.path.insert(0, "/opt/trn_rl_repo")

import numpy as np
import ml_dtypes

import concourse.bass as bass
import concourse.bacc as bacc
import concourse.mybir as mybir
import concourse.tile as tile
from concourse.bass_utils import run_bass_kernel_spmd

B, F = 1024, 2048
NK, KDIM = 32, 8
KD = NK * KDIM  # 256
NCORES = 8
RB = B // NCORES  # 128 rows per core
FOUT = F + NK  # 2080

N_WARM_MM = 24


def act_offload(g, j, c):
    """Which chunk-1 relu passes run on ACT instead of DVE (load balance)."""
    if c != 1:
        return False
    if g >= NK - 2:
        return False  # keep ACT free near the end so exp isn't the tail
    return j == 1 or (j == 3 and g % 2 == 0)


def emit_kernel(nc, tc, xt_ap, x_ap, w_ap, out_ap):
    f32 = mybir.dt.float32
    bf16 = mybir.dt.bfloat16
    AF = mybir.ActivationFunctionType
    ALU = mybir.AluOpType

    # Constants embedded in the NEFF.
    ident_np = np.eye(128, dtype=np.float32)
    l0 = np.zeros((128, NK), np.float32)
    l0[np.arange(128), np.arange(128) // KDIM] = 1.0  # kd 0..127 -> k 0..15
    l1 = np.zeros((128, NK), np.float32)
    l1[np.arange(128), 16 + np.arange(128) // KDIM] = 1.0  # kd 128..255 -> k 16..31
    ident_d = nc.inline_tensor(ident_np, name="ident_c")
    # All bf16 selector constants in ONE inline tensor / one DMA:
    # [lhs0 | lhs1 | lhs1x2 | lhs0m2 | lhs0rep4 | lhs1n_rep4]
    combo_np = np.concatenate(
        [l0, l1, 2 * l1, -2 * l0, np.tile(l0, (1, 4)), np.tile(-l1, (1, 4))], axis=1
    )
    combo_d = nc.inline_tensor(combo_np.astype(ml_dtypes.bfloat16), name="combo_c")

    with tc.tile_pool(name="persist", bufs=1) as pp:
        # ---- constants: one combined DMA on the sync queue; the gpsimd
        # queue starts the W transfer immediately.
        combo = pp.tile([128, 4 * NK + 256], bf16, name="combo")
        nc.gpsimd.dma_start(combo[:], combo_d.ap())
        lhs0 = combo[:, 0:NK]
        lhs1 = combo[:, NK : 2 * NK]
        lhs1x2 = combo[:, 2 * NK : 3 * NK]
        lhs0m2 = combo[:, 3 * NK : 4 * NK]
        lhs0rep = combo[:, 4 * NK : 4 * NK + 128]
        lhs1nrep = combo[:, 4 * NK + 128 : 4 * NK + 256]
        ident = pp.tile([128, 128], f32, name="ident")
        nc.gpsimd.dma_start(ident[:], ident_d.ap())

        # Local-query m^T scalars (f32 copies of the bf16-rounded values so
        # self-pairs give exactly L1=0) + the full gathered-equivalent M_T.
        mTloc0f = pp.tile([128, RB], f32, name="mTloc0f")
        mTloc1f = pp.tile([128, RB], f32, name="mTloc1f")
        mTloc1n = pp.tile([128, RB], f32, name="mTloc1n")  # negated, ACT bias
        MT0 = pp.tile([128, B], bf16, name="MT0")
        MT1 = pp.tile([128, B], bf16, name="MT1")
        # mb accumulator: row p = 32*(i%4) + k, col g = i//4  (i = query row)
        mbcols = pp.tile([128, NK], f32, name="mbcols")

        # ---------------- Stage A: full m^T = W^T @ x^T (local) ----------
        with (
            tc.tile_pool(name="sa1", bufs=1) as sa1,
            tc.tile_pool(name="wps", bufs=1, space="PSUM") as wps,
            tc.tile_pool(name="mps", bufs=1, space="PSUM") as mps,
        ):
            # PE warm-up: junk matmuls so the HAM clock-gate is open
            # (2.4 GHz) when the m chains issue.
            junkin = sa1.tile([128, 384], bf16, name="junkin")
            nc.vector.memset(junkin[:], 0.0)
            junk = wps.tile([128, 384], f32, name="junk")
            for _ in range(N_WARM_MM):
                nc.tensor.matmul(
                    junk[:], junkin[:, 0:128], junkin[:], start=True, stop=True
                )

            # W and x^T (bf16, host-tiled, fully contiguous) striped across
            # the two ~185 GB/s hardware DGE rings (scalar=q10, sync=q1),
            # W quarters interleaved with x chunks in consumption order.
            wsbb = sa1.tile([128, 16 * KD], bf16, name="wsbb")
            xts = sa1.tile([128, 16 * B], bf16, name="xts")
            xts_v = xts[:].rearrange("p (i b) -> p i b", i=16)

            def wq(j):
                sl = slice(j * 4 * KD, (j + 1) * 4 * KD)
                return wsbb[:, sl], w_ap[:, sl]

            def xq(q):
                sl = slice(q * 2 * B, (q + 1) * 2 * B)
                return xts[:, sl], xt_ap[:, sl]

            for eng, pairs in (
                (nc.scalar, (wq(0), xq(0), wq(2), xq(2), xq(4), xq(6))),
                (nc.sync, (wq(1), xq(1), wq(3), xq(3), xq(5), xq(7))),
            ):
                for dst, src in pairs:
                    eng.dma_start(dst, src)

            # ACT exp-table preload (one-time ~2.7us; after the DMA issues
            # so it doesn't delay the scalar queue's transfers).
            warm_sb = sa1.tile([1, 2], f32, name="warm_sb")
            nc.vector.memset(warm_sb[:], 0.0)
            nc.scalar.activation(warm_sb[:], warm_sb[:], AF.Exp, scale=-1.0)

            m0 = mps.tile([128, B], f32, name="m0")
            m1 = mps.tile([128, B], f32, name="m1")
            for i in range(16):
                for hh in range(2):
                    sl = slice(hh * 512, (hh + 1) * 512)
                    nc.tensor.matmul(
                        m0[:, sl], wsbb[:, i * KD : i * KD + 128], xts_v[:, i, sl],
                        start=(i == 0), stop=(i == 15),
                    )
                    nc.tensor.matmul(
                        m1[:, sl], wsbb[:, i * KD + 128 : (i + 1) * KD], xts_v[:, i, sl],
                        start=(i == 0), stop=(i == 15),
                    )
                if i < 14:
                    # No-dep junk matmuls between DMA-gated chain segments:
                    # they soak up PE-idle so the HAM clock gate stays open
                    # (chains otherwise run at 1.2 GHz after any >3.4us wait).
                    nc.tensor.matmul(
                        junk[:], junkin[:, 0:128], junkin[:], start=True, stop=True
                    )

            # Host staging rotates the b2 axis per core so the local queries
            # always sit at columns 0..RB (the b2 sum is rotation-invariant).
            nc.vector.tensor_copy(MT0[:], m0[:])  # f32 PSUM -> bf16 SBUF
            nc.vector.tensor_copy(MT1[:], m1[:])
            nc.vector.tensor_copy(mTloc0f[:], MT0[:, 0:RB])  # bf16 -> f32 exact
            nc.vector.tensor_copy(mTloc1f[:], MT1[:, 0:RB])
            nc.vector.tensor_scalar(mTloc1n[:], mTloc1f[:], -1.0, None, ALU.mult)

        # ---------------- Stage B: pairwise L1 -> exp -> sum --------------
        # L1[b1,k,b2] = sum_d |a_d - s_d| = SA[k,b2] + SS[k,b1] - 2*sum_d
        # min(a_d, s_d)   (a = M_T column b2, s = local query column b1).
        # Chunk 0 uses |a-s| = a + s - 2*min(a,s); chunk 1 uses
        # |a-s| = (s-a) + 2*relu(a-s), so SA4 = SA_c0 - SA_c1.
        with tc.tile_pool(name="pre", bufs=1) as pre:
            # SA[k, b2] = sum_{d in k} M_T[kd, b2]. Stored as float32r and
            # injected into PSUM via a replicating rep4 matmul (k -> 32j+k)
            # so PE writes all 128 partitions at full rate with start=True.
            f32r = mybir.dt.float32r
            SA4 = pre.tile([128, B], f32r, name="SA4")
            identr = pre.tile([128, 128], f32r, name="identr")
            nc.scalar.copy(identr[:], ident[:])
            SS4n = pre.tile([128, NK], f32, name="SS4n")
            with tc.tile_pool(name="prep", bufs=1, space="PSUM") as prep:
                # SA prep at full 128 rows via 4x-replicated selectors, so
                # the PSUM->SBUF copy happens once.
                saps = prep.tile([128, B], f32, name="saps")
                for hh in range(2):
                    sl = slice(hh * 512, (hh + 1) * 512)
                    nc.tensor.matmul(
                        saps[:, sl], lhs0rep, MT0[:, sl], start=True, stop=False
                    )
                    nc.tensor.matmul(
                        saps[:, sl], lhs1nrep, MT1[:, sl], start=False, stop=True
                    )
                # Copy on ACT (idle here) so prep stays off the DVE path.
                nc.scalar.copy(SA4[:], saps[:])
                # SS4n[32j+k, g] = -SS[k, 4g+j] = -sum_{d in k} mTloc[kd, 4g+j]
                ssps = prep.tile([32, RB], f32, name="ssps")
                nc.tensor.matmul(
                    ssps[:], lhs0, MT0[:, 0:RB], start=True, stop=False
                )
                nc.tensor.matmul(
                    ssps[:], lhs1, MT1[:, 0:RB], start=False, stop=True
                )
                ssn = pre.tile([32, RB], f32, name="ssn")
                nc.scalar.activation(
                    ssn[:], ssps[:], AF.Copy, bias=0.0, scale=-1.0
                )
                # SS4n[32j + k, g] = ssn[k, 4g + j], one strided DMA per j
                ssn_v = ssn[:].rearrange("k (g j) -> k g j", j=4)
                for j in range(4):
                    nc.sync.dma_start(
                        SS4n[32 * j : 32 * j + 32, :], ssn_v[:, :, j]
                    )

            def flush_mb(finp, half):
                # mbcols[32*j + k, g] holds mb for row i = 4*g + j, kernel k.
                gs = slice(16 * half, 16 * half + 16)
                mbT = finp.tile([16, 128], f32, name="mbT")
                nc.tensor.transpose(mbT[:], mbcols[:, gs], ident[:])
                mbTs = pp.tile([16, 128], f32, name="mbTs")
                nc.vector.tensor_copy(mbTs[:], mbT[:])
                # out[4g + j, F + k] = mbTs[g - 16*half, 32j + k]
                ov = out_ap[:, F:FOUT].rearrange("(g j) k -> g j k", j=4)
                nc.sync.dma_start(
                    ov[gs], mbTs[:].rearrange("g (j k) -> g j k", j=4)
                )

            with (
                tc.tile_pool(name="ab", bufs=20) as ab,
                tc.tile_pool(name="pb", bufs=3, space="PSUM") as pb,
                tc.tile_pool(name="ep", bufs=3) as ep,
                tc.tile_pool(name="finp", bufs=2, space="PSUM") as finp,
            ):
                for g in range(NK):
                    if g == 8:
                        # Pass x through to out[:, :F], staged through ab-pool
                        # slots: the pool's slot-reuse deps delay these DMAs
                        # into stage B, after the xT/W loads are done.
                        for pc in range(4):
                            xstg = ab.tile([128, 512], f32, name="a")
                            cs = slice(pc * 512, (pc + 1) * 512)
                            nc.gpsimd.dma_start(xstg[:], x_ap[:, cs])
                            nc.gpsimd.dma_start(out_ap[:, cs], xstg[:])
                    if g == 20:
                        flush_mb(finp, 0)
                    pg = pb.tile([128, B], f32, name="pg")
                    # Init PSUM with the SA term via a PE identity matmul
                    # (start=True sets has_written; a non-PE write would be
                    # overwritten by the first accumulating matmul).
                    for hh in range(2):
                        sl = slice(hh * 512, (hh + 1) * 512)
                        nc.tensor.matmul(
                            pg[:, sl], identr[:], SA4[:, sl],
                            start=True, stop=False,
                            skip_group_check=True,
                        )
                    for c, (MTc, lhsc) in enumerate(
                        ((MT0, lhs0m2), (MT1, lhs1x2))
                    ):
                        atiles = []
                        for j in range(4):
                            i = 4 * g + j
                            a = ab.tile([128, B], bf16, name="a")
                            if c == 0:
                                # min(a, s) in one single-op DVE pass; the
                                # -2 factor lives in the lhs0m2 selector.
                                nc.vector.tensor_scalar(
                                    a[:], MTc[:], mTloc0f[:, i : i + 1], None,
                                    ALU.min,
                                )
                            elif act_offload(g, j, c):
                                nc.scalar.activation(
                                    a[:], MTc[:], AF.Relu,
                                    bias=mTloc1n[:, i : i + 1], scale=1.0,
                                )
                            else:
                                nc.vector.tensor_scalar(
                                    a[:], MTc[:], mTloc1f[:, i : i + 1], 0.0,
                                    ALU.subtract, ALU.max,
                                )
                            atiles.append(a)
                        # j-inner: consecutive matmuls target distinct column
                        # tiles of the PE array and run ~concurrently.
                        for hh in range(2):
                            sl = slice(hh * 512, (hh + 1) * 512)
                            for j in range(4):
                                nc.tensor.matmul(
                                    pg[32 * j : 32 * j + 32, sl],
                                    lhsc, atiles[j][:, sl],
                                    start=False, stop=(c == 1),
                                    tile_position=(0, 32 * j),
                                    skip_group_check=True,
                                )
                    es = ep.tile([128, B], bf16, name="es")
                    nc.scalar.activation(
                        es[:], pg[:], AF.Exp, scale=-1.0,
                        bias=SS4n[:, g : g + 1],
                        accum_out=mbcols[:, g : g + 1],
                    )

                flush_mb(finp, 1)


def build_program():
    nc = bacc.Bacc("TRN2", num_devices=NCORES)
    # xT_t[p, i*B + b]  = x[b, i*128 + p]   (bf16, host-tiled, contiguous)
    # W_t[p, i*KD + k]  = W[i*128 + p, k]   (bf16, host-tiled, contiguous)
    xt = nc.dram_tensor("xT_t", [128, 16 * B], mybir.dt.bfloat16, kind="ExternalInput")
    x_sh = nc.dram_tensor("x_shard", [RB, F], mybir.dt.float32, kind="ExternalInput")
    w = nc.dram_tensor("W_t", [128, 16 * KD], mybir.dt.bfloat16, kind="ExternalInput")
    out = nc.dram_tensor("out_shard", [RB, FOUT], mybir.dt.float32, kind="ExternalOutput")
    with tile.TileContext(nc, num_cores=NCORES) as tc:
        emit_kernel(nc, tc, xt.ap(), x_sh.ap(), w.ap(), out.ap())
    nc.compile()
    return nc


def make_in_maps(x, W):
    x = np.ascontiguousarray(np.asarray(x, dtype=np.float32))
    W = np.ascontiguousarray(np.asarray(W, dtype=np.float32))
    # [F, B] -> [128, 16*B] tiled: xt_t[p, i*B + b] = x[b, i*128 + p]
    xt = x.T.astype(ml_dtypes.bfloat16)
    w_t = np.ascontiguousarray(
        W.astype(ml_dtypes.bfloat16).reshape(16, 128, KD).transpose(1, 0, 2).reshape(128, 16 * KD)
    )
    maps = []
    for c in range(NCORES):
        # roll so core c's own query rows sit at b2 columns 0..RB
        xt_c = np.roll(xt, -c * RB, axis=1)
        xt_t = np.ascontiguousarray(
            xt_c.reshape(16, 128, B).transpose(1, 0, 2).reshape(128, 16 * B)
        )
        maps.append({"xT_t": xt_t, "x_shard": x[c * RB : (c + 1) * RB], "W_t": w_t})
    return maps


def kernel(x, W):
    x = np.ascontiguousarray(np.asarray(x, dtype=np.float32))
    W = np.ascontiguousarray(np.asarray(W, dtype=np.float32))
    assert x.shape == (B, F) and W.shape == (F, KD)
    nc = build_program()
    res = run_bass_kernel_spmd(nc, make_in_maps(x, W), core_ids=list(range(NCORES)))
    out = np.concatenate(
        [res.results[c]["out_shard"] for c in range(NCORES)], axis=0
    )
    return out.astype(np.float32)
